# revision 20
# baseline (speedup 1.0000x reference)
"""LSTM ActionEncoder kernel for Trainium2 (8 NeuronCores, data-parallel on batch).

Reference computation (T=20, B=2048, IN=512, H=1024):
    x_emb = obs @ W_emb.T + b_emb                      # [T, B, 512]
    per step: gates = x_t @ W_ih.T + h @ W_hh.T + b    # LSTM cell, i,f,g,o
    returns h_final                                    # [B, 1024]

Device algorithm (per core, B=256) — per-gate mixed precision, batch-major,
stationary-reuse:
  * Embedding + input projection folded on the host into one [3 -> 4096] map
    applied to (obs_x, obs_y, 1) per step, streamed as row-tiled K=32 fp16
    matmuls (stationary = obs rows, moving = folded weights).
  * The recurrent matmul is split per gate by numerical sensitivity
    (measured: HW DoubleRow streams at 1 col/cycle, i.e. 2x bf16 MACs, so
    fp8 + full residual compensation would cost as much as fp16):
      - F16_GATES stream in plain fp16 (stationary = h^T fp16 128-chunk,
        moving = fp16 W columns, N=512) — exact, 2x the stream time.
      - The rest run fp8(e4m3) MatmulPerfMode.DoubleRow (stationary =
        h'^T value pair-tiles [128, 2, 128], moving = packed W slices
        [128, 2, 256], 256-deep contraction), optionally with fp8 residual
        streams for W (extra moving streams, same stationary) and/or h
        (same W streams against a stationary r8 pair).
    The tanh gate (g) is the most sensitive (activation slope 1 vs the
    sigmoid gates' effective 1/4), the input gate the least.
  * Stationary operands are reused across consecutive streams: tile
    legalization emits one LDWEIGHTS per matmul, so a post-compile pass
    drops sync-free LDWEIGHTS that reload the identical stationary
    (~133 ns of pure PE dead time each).
  * All four gate activations collapse into ONE wide tanh per PSUM group:
    sigmoid(x) = (tanh(x/2)+1)/2 with the x/2 pre-folded into the i,f,o
    weight columns on the host; the affine fixups fuse into DVE
    scalar_tensor_tensor ops on a doubled state (c2 = 2c, h' = 2h; the 2x
    cancels inside the folded weight scale, the final h is halved on host).
  * PSUM gate scale S=64 (clears fp8 denormals in W); one ACT tanh with
    scale=1/S rescales and activates in a single pass.
  * h' is produced as fp16 [batch, hdim]; 128x128 XBAR DMA transposes give
    h'^T fp16 (the fp16 stationary), and the Pool engine derives the fp8
    value (+ residual) tiles for the fp8 gates' stationaries.

Layouts (per core; bc = batch half, hh = hdim half, kp = hdim 256-pair,
i = which 128 inside the pair, kc = hdim 128-chunk, b' = batch 0..127,
col = gate row 0..4095, col16 = compact column index over F16_GATES):
  wpk/wrk [4,2,128,4096]: [kp, i, p, col] = W8[col, 256*kp+128*i+p]
  w16k    [8,128,1024*n16]: [kc, p, col16] = W16[col, 128*kc+p]
  wfold   [128,4096]: rows 32r+{0,1,2} = scaled (Wf_x, Wf_y, bias)[col]
  obs     [20,128,256]: rows 32r+{0,1} = obs_x, obs_y, rows 32r+2 = 1.0
  h16t    [128,2,4,2,128] f16: [p, bc, kp, i, b'] = 2h^T
  h8t/r8t [128,2,4,2,128] f8: same, quantized (value, residual)
  c0t     [128,2,1024]: [p, bc, hd] = 2*c0[128*bc+p, hd]
  hout    [128,2,1024] f16: h' of the last step (host halves; no transpose)
"""

import numpy as np
import ml_dtypes

T = 20
H = 1024
NCORES = 8
B = 256  # batch per core
S = 64.0  # psum gate scale

# gate order i,f,g,o = 0,1,2,3
F16_GATES = (2,)  # stream these gates in fp16
WRES_GATES = ()  # fp8 gates with a W-residual stream
HRES_GATES = ()  # fp8 gates with an h-residual stream

_CACHED_NC = None
LAST_RESULT = None  # BassKernelResults of the most recent run (for test harness)

E4M3 = ml_dtypes.float8_e4m3
N16 = len(F16_GATES)


def _dedupe_ldweights(nc):
    """Tile legalization emits one InstLdweights per matmul; this kernel
    reuses the same stationary across runs of consecutive streams, so the
    repeats are pure PE dead time (~133 ns each). Drop an InstLdweights
    when it reloads exactly what is already in the array and it carries no
    semaphore ops (so no sync counts change)."""
    import concourse.mybir as mybir

    PE = mybir.EngineType.PE

    def wkey(inst):
        pap = inst.ins[0]
        return (
            pap.memref,
            pap.offset,
            str(pap.ap),
            str(pap.dtype),
            str(inst.perf_mode),
            str(inst.is_transpose),
            str(inst.tile_position),
            str(inst.tile_size),
        )

    total_removed = 0
    for f in nc.m.functions:
        for blk in f.blocks:
            last = None
            kept = []
            removed = 0
            for inst in blk.instructions:
                if inst.engine != PE:
                    kept.append(inst)
                    continue
                tn = type(inst).__name__
                if tn == "InstLdweights":
                    k = wkey(inst)
                    si = inst.sync_info
                    sync_free = si is None or (
                        len(si.on_wait) == 0 and len(si.on_update) == 0
                    )
                    if k == last and sync_free:
                        removed += 1
                        continue
                    last = k
                    kept.append(inst)
                elif tn == "InstMatmult":
                    kept.append(inst)
                else:
                    last = None
                    kept.append(inst)
            if removed:
                blk.instructions = kept
                total_removed += removed
    return total_removed


def _build_program():
    import concourse.mybir as mybir
    import concourse.tile as tile
    from concourse import bacc

    f32 = mybir.dt.float32
    f16 = mybir.dt.float16
    f8 = mybir.dt.float8e4
    AFT = mybir.ActivationFunctionType
    ALU = mybir.AluOpType
    DR = mybir.MatmulPerfMode.DoubleRow

    f8_gates = tuple(g for g in range(4) if g not in F16_GATES)
    any_wres = len(WRES_GATES) > 0
    any_hres = len(HRES_GATES) > 0

    nc = bacc.Bacc("TRN2", target_bir_lowering=False)
    wpk_dram = nc.dram_tensor("wpk", (4, 2, 128, 4096), f8, kind="ExternalInput")
    if any_wres:
        wrk_dram = nc.dram_tensor("wrk", (4, 2, 128, 4096), f8, kind="ExternalInput")
    if N16:
        w16_dram = nc.dram_tensor(
            "w16k", (8, 128, 1024 * N16), f16, kind="ExternalInput"
        )
    wfold_dram = nc.dram_tensor("wfold", (128, 4096), f16, kind="ExternalInput")
    obs_dram = nc.dram_tensor("obs", (T, 128, B), f16, kind="ExternalInput")
    h16_dram = nc.dram_tensor("h16t", (128, 2, 4, 2, 128), f16, kind="ExternalInput")
    h0_dram = nc.dram_tensor("h8t", (128, 2, 4, 2, 128), f8, kind="ExternalInput")
    if any_hres:
        r0_dram = nc.dram_tensor("r8t", (128, 2, 4, 2, 128), f8, kind="ExternalInput")
    c0_dram = nc.dram_tensor("c0t", (128, 2, 1024), f16, kind="ExternalInput")
    out_dram = nc.dram_tensor("hout", (128, 2, 1024), f16, kind="ExternalOutput")

    with tile.TileContext(nc) as tc:
        with (
            tc.tile_pool(name="wpool", bufs=1) as wpool,
            tc.tile_pool(name="spool", bufs=1) as spool,
            tc.tile_pool(name="opool", bufs=2) as opool,
            tc.tile_pool(name="hpool", bufs=2) as hpool,
            tc.tile_pool(name="gpool", bufs=3) as gpool,
            tc.tile_pool(name="upool", bufs=3) as upool,
            tc.tile_pool(name="ppool", bufs=2, space="PSUM") as ppool,
        ):
            wfold_sb = wpool.tile([128, 4096], f16, name="wfold_sb")
            nc.sync.dma_start(wfold_sb[:], wfold_dram[:])

            ht16 = [
                spool.tile([128, 2, 4, 2, 128], f16, name=f"ht16_{s}")
                for s in range(2)
            ]
            ht8 = [
                spool.tile([128, 2, 4, 2, 128], f8, name=f"ht8_{s}") for s in range(2)
            ]
            if any_hres:
                rt8 = [
                    spool.tile([128, 2, 4, 2, 128], f8, name=f"rt8_{s}")
                    for s in range(2)
                ]
            c2 = spool.tile([128, 2, 1024], f16, name="c2")
            nc.scalar.dma_start(ht16[0][:], h16_dram[:])
            nc.scalar.dma_start(ht8[0][:], h0_dram[:])
            if any_hres:
                nc.scalar.dma_start(rt8[0][:], r0_dram[:])
            nc.scalar.dma_start(c2[:], c0_dram[:])

            w_sb = wpool.tile([128, 4, 2, 4096], f8, name="w_sb")
            if any_wres:
                wr_sb = wpool.tile([128, 4, 2, 4096], f8, name="wr_sb")
            if N16:
                w16_sb = wpool.tile([128, 8, 1024 * N16], f16, name="w16_sb")
            dma_engs = [nc.sync, nc.scalar, nc.gpsimd]
            di = 0
            for kp in range(4):
                for i in range(2):
                    dma_engs[di % 3].dma_start(w_sb[:, kp, i], wpk_dram[kp, i])
                    di += 1
                    if any_wres:
                        dma_engs[di % 3].dma_start(wr_sb[:, kp, i], wrk_dram[kp, i])
                        di += 1
                    if N16:
                        kc = 2 * kp + i
                        dma_engs[di % 3].dma_start(w16_sb[:, kc], w16_dram[kc])
                        di += 1

            # per-bank matmul count (for stop bookkeeping)
            def bank_mm(g):
                if g in F16_GATES:
                    return 1 + 8
                return 1 + 4 * (2 + 2 * (g in WRES_GATES) + 2 * (g in HRES_GATES))

            n_mm = [bank_mm(g) for g in range(4)]

            for t in range(T):
                rd, wr = t % 2, (t + 1) % 2
                obs_t = opool.tile([128, B], f16, name="obs_t", tag="obs_t")
                nc.gpsimd.dma_start(obs_t[:], obs_dram[t])
                h16 = hpool.tile([128, 2, 1024], f16, name="h16", tag="h16")
                groups = [(0, 0), (0, 1), (1, 0), (1, 1)]
                for pi in range(2):
                    pair = groups[2 * pi : 2 * pi + 2]
                    # x-parts of both groups first: h-independent PE work
                    # that bridges the recurrent-dependency step boundary
                    pstiles = {}
                    for bc, hh in pair:
                        pstiles[(bc, hh)] = ppool.tile(
                            [128, 4, 512], f32, name="ps", tag="ps"
                        )
                    # gate-major so the two groups' identical obs
                    # stationaries are adjacent (LDWEIGHTS dedup)
                    for g in range(4):
                        r = g
                        for bc, hh in pair:
                            nc.tensor.matmul(
                                pstiles[(bc, hh)][:, g, :],
                                obs_t[32 * r : 32 * r + 32, 128 * bc : 128 * bc + 128],
                                wfold_sb[
                                    32 * r : 32 * r + 32,
                                    1024 * g + 512 * hh : 1024 * g + 512 * hh + 512,
                                ],
                                start=True,
                                stop=False,
                                tile_position=(32 * r, 0),
                            )
                    # fp8 DoubleRow gates: both groups of the pair inside one
                    # kp loop, so the second group reuses the just-loaded h8
                    # stationary (LDWEIGHTS dedup)
                    cnt = {(hh_, g): 1 for _, hh_ in pair for g in range(4)}
                    for kp in range(4):
                        for wsb_, gates in (
                            (w_sb, f8_gates),
                            (wr_sb, WRES_GATES) if any_wres else (None, ()),
                        ):
                            for g in gates:
                                for bc, hh in pair:
                                    for q in range(2):
                                        c0_ = 1024 * g + 512 * hh + 256 * q
                                        cnt[(hh, g)] += 1
                                        nc.tensor.matmul(
                                            pstiles[(bc, hh)][
                                                :, g, 256 * q : 256 * q + 256
                                            ],
                                            ht8[rd][:, bc, kp],
                                            wsb_[:, kp, :, c0_ : c0_ + 256],
                                            start=False,
                                            stop=(cnt[(hh, g)] == n_mm[g]),
                                            perf_mode=DR,
                                        )
                        for g in HRES_GATES:
                            for bc, hh in pair:
                                for q in range(2):
                                    c0_ = 1024 * g + 512 * hh + 256 * q
                                    cnt[(hh, g)] += 1
                                    nc.tensor.matmul(
                                        pstiles[(bc, hh)][:, g, 256 * q : 256 * q + 256],
                                        rt8[rd][:, bc, kp],
                                        w_sb[:, kp, :, c0_ : c0_ + 256],
                                        start=False,
                                        stop=(cnt[(hh, g)] == n_mm[g]),
                                        perf_mode=DR,
                                    )
                    for bc, hh in pair:
                        ps = pstiles[(bc, hh)]
                        tt = gpool.tile([128, 4, 512], f16, name="tt", tag="tt")
                        # fp16 gates (per group, so the first group's banks
                        # finish and its ACT span fires while the second
                        # group's fp16 streams still run)
                        for kc in range(8):
                            for gi, g in enumerate(F16_GATES):
                                c16 = 1024 * gi + 512 * hh
                                cnt[(hh, g)] += 1
                                nc.tensor.matmul(
                                    ps[:, g, :],
                                    ht16[rd][:, bc, kc // 2, kc % 2, :],
                                    w16_sb[:, kc, c16 : c16 + 512],
                                    start=False,
                                    stop=(cnt[(hh, g)] == n_mm[g]),
                                )
                        # all four gates in one wide tanh (sigma via tanh fold)
                        nc.scalar.activation(tt[:], ps[:], AFT.Tanh, scale=1.0 / S)
                        u = upool.tile([128, 512], f16, name="u", tag="u")
                        v = upool.tile([128, 512], f16, name="v", tag="v")
                        th = upool.tile([128, 512], f16, name="th", tag="th")
                        csl = c2[:, bc, 512 * hh : 512 * hh + 512]
                        # u = (t_i + 1) * t_g ; v = (t_f + 1) * c2 ; c2' = v/2 + u
                        nc.vector.scalar_tensor_tensor(
                            u, tt[:, 0], 1.0, tt[:, 2], ALU.add, ALU.mult
                        )
                        nc.vector.scalar_tensor_tensor(
                            v, tt[:, 1], 1.0, csl, ALU.add, ALU.mult
                        )
                        nc.vector.scalar_tensor_tensor(
                            csl, v, 0.5, u, ALU.mult, ALU.add
                        )
                        # th = tanh(c) = tanh(0.5 * c2') ; h' = (t_o + 1) * th
                        nc.scalar.activation(th, csl, AFT.Tanh, scale=0.5)
                        nc.vector.scalar_tensor_tensor(
                            h16[:, bc, 512 * hh : 512 * hh + 512],
                            tt[:, 3],
                            1.0,
                            th,
                            ALU.add,
                            ALU.mult,
                        )
                        if t < T - 1:
                            # h'^T via one XBAR DMA per group (tile-major
                            # 128x128 blocks land exactly on the [kp,i,b']
                            # layout); fp8 value (+ residual) on Pool
                            nc.sync.dma_start(
                                ht16[wr][:, bc, 2 * hh : 2 * hh + 2, :, :],
                                h16[:, bc, 512 * hh : 512 * hh + 512],
                                transpose=True,
                            )
                            for kp in (2 * hh, 2 * hh + 1):
                                tsl = (slice(None), bc, kp)
                                nc.gpsimd.tensor_copy(ht8[wr][tsl], ht16[wr][tsl])
                                if any_hres:
                                    nc.gpsimd.tensor_sub(
                                        rt8[wr][tsl], ht16[wr][tsl], ht8[wr][tsl]
                                    )

                if t == T - 1:
                    nc.sync.dma_start(out_dram[:], h16[:])

    nc.compile()
    return nc


def _host_prep(inputs):
    obs = np.asarray(inputs["obs_traj"], dtype=np.float32)
    h0 = np.asarray(inputs["h0"], dtype=np.float32)
    c0 = np.asarray(inputs["c0"], dtype=np.float32)
    W_emb = np.asarray(inputs["W_emb"], dtype=np.float32)
    b_emb = np.asarray(inputs["b_emb"], dtype=np.float32)
    W_ih = np.asarray(inputs["W_ih"], dtype=np.float32)
    W_hh = np.asarray(inputs["W_hh"], dtype=np.float32)
    b_ih = np.asarray(inputs["b_ih"], dtype=np.float32)
    b_hh = np.asarray(inputs["b_hh"], dtype=np.float32)

    gs = np.full(4 * H, 0.5, np.float32)  # sigma gates get the tanh half-angle
    gs[2 * H : 3 * H] = 1.0
    Wf = (W_ih @ W_emb).astype(np.float32)  # [4096, 2]
    biasf = (W_ih @ b_emb + b_ih + b_hh).astype(np.float32)

    # packed recurrent weights (psum = S * gate_mod; h' = 2h)
    Wp = S * gs[:, None] * W_hh / 2.0  # [4096, 1024]
    W8 = Wp.astype(E4M3)
    Wr8 = (Wp - W8.astype(np.float32)).astype(E4M3)

    def pack_w(Wq):  # [4096,1024] -> [4,2,128,4096]: [kp,i,p,col]
        return np.ascontiguousarray(Wq.T.reshape(4, 2, 128, 4096))

    wpk = pack_w(W8)
    wrk = pack_w(Wr8)
    if N16:
        cols16 = np.concatenate(
            [np.arange(1024 * g, 1024 * g + 1024) for g in F16_GATES]
        )
        # [kc, p, col16] = W16[col, 128*kc+p]
        w16k = np.ascontiguousarray(
            Wp[cols16].astype(np.float16).T.reshape(8, 128, 1024 * N16)
        )

    wfold = np.zeros((128, 4096), np.float16)
    for r in range(4):
        wfold[32 * r + 0] = (S * gs * Wf[:, 0]).astype(np.float16)
        wfold[32 * r + 1] = (S * gs * Wf[:, 1]).astype(np.float16)
        wfold[32 * r + 2] = (S * gs * biasf).astype(np.float16)

    def pack_hT(x):  # [256,1024] f32 -> [128,2,4,2,128]: [p,bc,kp,i,b']
        return np.ascontiguousarray(
            x.T.reshape(4, 2, 128, 2, 128).transpose(2, 3, 0, 1, 4)
        )

    in_maps = []
    for c in range(NCORES):
        sl = slice(B * c, B * (c + 1))
        obs_aug = np.zeros((T, 128, B), np.float16)
        for r in range(4):
            obs_aug[:, 32 * r + 0] = obs[:, sl, 0]
            obs_aug[:, 32 * r + 1] = obs[:, sl, 1]
            obs_aug[:, 32 * r + 2] = 1.0
        hp = pack_hT(2.0 * h0[sl])
        h16t = hp.astype(np.float16)
        h8t = h16t.astype(E4M3)  # device quantizes fp16 h, match that
        r8t = (h16t.astype(np.float32) - h8t.astype(np.float32)).astype(E4M3)
        c0t = np.ascontiguousarray(
            (2.0 * c0[sl]).reshape(2, 128, 1024).transpose(1, 0, 2)
        ).astype(np.float16)
        m = dict(wpk=wpk, wfold=wfold, obs=obs_aug, h16t=h16t, h8t=h8t, c0t=c0t)
        if len(WRES_GATES) > 0:
            m["wrk"] = wrk
        if len(HRES_GATES) > 0:
            m["r8t"] = r8t
        if N16:
            m["w16k"] = w16k
        in_maps.append(m)
    return in_maps


def _unpack_out(o):  # [128,2,1024] f16 -> [256,1024] f32 (h = h'/2)
    return 0.5 * o.astype(np.float32).transpose(1, 0, 2).reshape(B, H)


def kernel(**inputs) -> np.ndarray:
    global _CACHED_NC, LAST_RESULT
    from concourse.bass_utils import run_bass_kernel_spmd

    in_maps = _host_prep(inputs)
    if _CACHED_NC is None:
        _CACHED_NC = _build_program()
        _dedupe_ldweights(_CACHED_NC)
    res = run_bass_kernel_spmd(_CACHED_NC, in_maps, core_ids=list(range(NCORES)))
    LAST_RESULT = res
    out = np.concatenate(
        [_unpack_out(res.results[c]["hout"]) for c in range(NCORES)], axis=0
    )
    return np.ascontiguousarray(out.astype(np.float32))


# revision 22
# speedup vs baseline: 1.0223x; 1.0223x over previous
"""LSTM ActionEncoder kernel for Trainium2 (8 NeuronCores, data-parallel on batch).

Reference computation (T=20, B=2048, IN=512, H=1024):
    x_emb = obs @ W_emb.T + b_emb                      # [T, B, 512]
    per step: gates = x_t @ W_ih.T + h @ W_hh.T + b    # LSTM cell, i,f,g,o
    returns h_final                                    # [B, 1024]

Device algorithm (per core, B=256) — per-gate mixed precision, batch-major,
stationary-reuse:
  * Embedding + input projection folded on the host into one [3 -> 4096] map
    applied to (obs_x, obs_y, 1) per step, streamed as row-tiled K=32 fp16
    matmuls (stationary = obs rows, moving = folded weights).
  * The recurrent matmul is split per gate by numerical sensitivity
    (measured: HW DoubleRow streams at 1 col/cycle, i.e. 2x bf16 MACs, so
    fp8 + full residual compensation would cost as much as fp16):
      - F16_GATES stream in plain fp16 (stationary = h^T fp16 128-chunk,
        moving = fp16 W columns, N=512) — exact, 2x the stream time.
      - The rest run fp8(e4m3) MatmulPerfMode.DoubleRow (stationary =
        h'^T value pair-tiles [128, 2, 128], moving = packed W slices
        [128, 2, 256], 256-deep contraction), optionally with fp8 residual
        streams for W (extra moving streams, same stationary) and/or h
        (same W streams against a stationary r8 pair).
    The tanh gate (g) is the most sensitive (activation slope 1 vs the
    sigmoid gates' effective 1/4), the input gate the least.
  * Stationary operands are reused across consecutive streams: tile
    legalization emits one LDWEIGHTS per matmul, so a post-compile pass
    drops sync-free LDWEIGHTS that reload the identical stationary
    (~133 ns of pure PE dead time each).
  * All four gate activations collapse into ONE wide tanh per PSUM group:
    sigmoid(x) = (tanh(x/2)+1)/2 with the x/2 pre-folded into the i,f,o
    weight columns on the host; the affine fixups fuse into DVE
    scalar_tensor_tensor ops on a doubled state (c2 = 2c, h' = 2h; the 2x
    cancels inside the folded weight scale, the final h is halved on host).
  * PSUM gate scale S=64 (clears fp8 denormals in W); one ACT tanh with
    scale=1/S rescales and activates in a single pass.
  * h' is produced as fp16 [batch, hdim]; 128x128 XBAR DMA transposes give
    h'^T fp16 (the fp16 stationary), and the Pool engine derives the fp8
    value (+ residual) tiles for the fp8 gates' stationaries.

Layouts (per core; bc = batch half, hh = hdim half, kp = hdim 256-pair,
i = which 128 inside the pair, kc = hdim 128-chunk, b' = batch 0..127,
col = gate row 0..4095, col16 = compact column index over F16_GATES):
  wpk/wrk [4,2,128,4096]: [kp, i, p, col] = W8[col, 256*kp+128*i+p]
  w16k    [8,128,1024*n16]: [kc, p, col16] = W16[col, 128*kc+p]
  wfold   [128,4096]: rows 32r+{0,1,2} = scaled (Wf_x, Wf_y, bias)[col]
  obs     [20,128,256]: rows 32r+{0,1} = obs_x, obs_y, rows 32r+2 = 1.0
  h16t    [128,2,4,2,128] f16: [p, bc, kp, i, b'] = 2h^T
  h8t/r8t [128,2,4,2,128] f8: same, quantized (value, residual)
  c0t     [128,2,1024]: [p, bc, hd] = 2*c0[128*bc+p, hd]
  hout    [128,2,1024] f16: h' of the last step (host halves; no transpose)
"""

import numpy as np
import ml_dtypes

T = 20
H = 1024
NCORES = 8
B = 256  # batch per core
S = 64.0  # psum gate scale

# gate order i,f,g,o = 0,1,2,3
F16_GATES = (2,)  # stream these gates in fp16
WRES_GATES = ()  # fp8 gates with a W-residual stream
HRES_GATES = ()  # fp8 gates with an h-residual stream

_CACHED_NC = None
LAST_RESULT = None  # BassKernelResults of the most recent run (for test harness)

E4M3 = ml_dtypes.float8_e4m3
N16 = len(F16_GATES)


def _dedupe_ldweights(nc):
    """Tile legalization emits one InstLdweights per matmul; this kernel
    reuses the same stationary across runs of consecutive streams, so the
    repeats are pure PE dead time (~133 ns each). Drop an InstLdweights
    when it reloads exactly what is already in the array and it carries no
    semaphore ops (so no sync counts change)."""
    import concourse.mybir as mybir

    PE = mybir.EngineType.PE

    def wkey(inst):
        pap = inst.ins[0]
        return (
            pap.memref,
            pap.offset,
            str(pap.ap),
            str(pap.dtype),
            str(inst.perf_mode),
            str(inst.is_transpose),
            str(inst.tile_position),
            str(inst.tile_size),
        )

    total_removed = 0
    for f in nc.m.functions:
        for blk in f.blocks:
            last = None
            kept = []
            removed = 0
            for inst in blk.instructions:
                if inst.engine != PE:
                    kept.append(inst)
                    continue
                tn = type(inst).__name__
                if tn == "InstLdweights":
                    k = wkey(inst)
                    si = inst.sync_info
                    sync_free = si is None or (
                        len(si.on_wait) == 0 and len(si.on_update) == 0
                    )
                    if k == last and sync_free:
                        removed += 1
                        continue
                    last = k
                    kept.append(inst)
                elif tn == "InstMatmult":
                    kept.append(inst)
                else:
                    last = None
                    kept.append(inst)
            if removed:
                blk.instructions = kept
                total_removed += removed
    return total_removed


def _build_program():
    import concourse.mybir as mybir
    import concourse.tile as tile
    from concourse import bacc

    f32 = mybir.dt.float32
    f16 = mybir.dt.float16
    f8 = mybir.dt.float8e4
    AFT = mybir.ActivationFunctionType
    ALU = mybir.AluOpType
    DR = mybir.MatmulPerfMode.DoubleRow

    f8_gates = tuple(g for g in range(4) if g not in F16_GATES)
    BANK = {0: 0, 2: 1, 1: 2, 3: 3}  # psum bank order i, g, f, o
    any_wres = len(WRES_GATES) > 0
    any_hres = len(HRES_GATES) > 0

    nc = bacc.Bacc("TRN2", target_bir_lowering=False)
    wpk_dram = nc.dram_tensor("wpk", (4, 2, 128, 4096), f8, kind="ExternalInput")
    if any_wres:
        wrk_dram = nc.dram_tensor("wrk", (4, 2, 128, 4096), f8, kind="ExternalInput")
    if N16:
        w16_dram = nc.dram_tensor(
            "w16k", (8, 128, 1024 * N16), f16, kind="ExternalInput"
        )
    wfold_dram = nc.dram_tensor("wfold", (128, 4096), f16, kind="ExternalInput")
    obs_dram = nc.dram_tensor("obs", (T, 128, B), f16, kind="ExternalInput")
    h16_dram = nc.dram_tensor("h16t", (128, 2, 4, 2, 128), f16, kind="ExternalInput")
    h0_dram = nc.dram_tensor("h8t", (128, 2, 4, 2, 128), f8, kind="ExternalInput")
    if any_hres:
        r0_dram = nc.dram_tensor("r8t", (128, 2, 4, 2, 128), f8, kind="ExternalInput")
    c0_dram = nc.dram_tensor("c0t", (128, 2, 1024), f16, kind="ExternalInput")
    out_dram = nc.dram_tensor("hout", (128, 2, 1024), f16, kind="ExternalOutput")

    with tile.TileContext(nc) as tc:
        with (
            tc.tile_pool(name="wpool", bufs=1) as wpool,
            tc.tile_pool(name="spool", bufs=1) as spool,
            tc.tile_pool(name="opool", bufs=2) as opool,
            tc.tile_pool(name="hpool", bufs=2) as hpool,
            tc.tile_pool(name="gpool", bufs=3) as gpool,
            tc.tile_pool(name="upool", bufs=3) as upool,
            tc.tile_pool(name="ppool", bufs=2, space="PSUM") as ppool,
        ):
            wfold_sb = wpool.tile([128, 4096], f16, name="wfold_sb")
            nc.sync.dma_start(wfold_sb[:], wfold_dram[:])

            ht16 = [
                spool.tile([128, 2, 4, 2, 128], f16, name=f"ht16_{s}")
                for s in range(2)
            ]
            ht8 = [
                spool.tile([128, 2, 4, 2, 128], f8, name=f"ht8_{s}") for s in range(2)
            ]
            if any_hres:
                rt8 = [
                    spool.tile([128, 2, 4, 2, 128], f8, name=f"rt8_{s}")
                    for s in range(2)
                ]
            c2 = spool.tile([128, 2, 1024], f16, name="c2")
            nc.scalar.dma_start(ht16[0][:], h16_dram[:])
            nc.scalar.dma_start(ht8[0][:], h0_dram[:])
            if any_hres:
                nc.scalar.dma_start(rt8[0][:], r0_dram[:])
            nc.scalar.dma_start(c2[:], c0_dram[:])

            w_sb = wpool.tile([128, 4, 2, 4096], f8, name="w_sb")
            if any_wres:
                wr_sb = wpool.tile([128, 4, 2, 4096], f8, name="wr_sb")
            if N16:
                w16_sb = wpool.tile([128, 8, 1024 * N16], f16, name="w16_sb")
            dma_engs = [nc.sync, nc.scalar, nc.gpsimd]
            di = 0
            for kp in range(4):
                for i in range(2):
                    dma_engs[di % 3].dma_start(w_sb[:, kp, i], wpk_dram[kp, i])
                    di += 1
                    if any_wres:
                        dma_engs[di % 3].dma_start(wr_sb[:, kp, i], wrk_dram[kp, i])
                        di += 1
                    if N16:
                        kc = 2 * kp + i
                        dma_engs[di % 3].dma_start(w16_sb[:, kc], w16_dram[kc])
                        di += 1

            # per-bank matmul count (for stop bookkeeping)
            def bank_mm(g):
                if g in F16_GATES:
                    return 1 + 8
                return 1 + 4 * (2 + 2 * (g in WRES_GATES) + 2 * (g in HRES_GATES))

            n_mm = [bank_mm(g) for g in range(4)]

            for t in range(T):
                rd, wr = t % 2, (t + 1) % 2
                obs_t = opool.tile([128, B], f16, name="obs_t", tag="obs_t")
                nc.gpsimd.dma_start(obs_t[:], obs_dram[t])
                h16 = hpool.tile([128, 2, 1024], f16, name="h16", tag="h16")
                groups = [(0, 0), (0, 1), (1, 0), (1, 1)]
                for pi in range(2):
                    pair = groups[2 * pi : 2 * pi + 2]
                    # x-parts of both groups first: h-independent PE work
                    # that bridges the recurrent-dependency step boundary
                    pstiles = {}
                    for bc, hh in pair:
                        pstiles[(bc, hh)] = ppool.tile(
                            [128, 4, 512], f32, name="ps", tag="ps"
                        )
                    # gate-major so the two groups' identical obs
                    # stationaries are adjacent (LDWEIGHTS dedup)
                    for g in range(4):
                        r = g
                        for bc, hh in pair:
                            nc.tensor.matmul(
                                pstiles[(bc, hh)][:, BANK[g], :],
                                obs_t[32 * r : 32 * r + 32, 128 * bc : 128 * bc + 128],
                                wfold_sb[
                                    32 * r : 32 * r + 32,
                                    1024 * g + 512 * hh : 1024 * g + 512 * hh + 512,
                                ],
                                start=True,
                                stop=False,
                                tile_position=(32 * r, 0),
                            )
                    for bc, hh in pair:
                        ps = pstiles[(bc, hh)]
                        tt = gpool.tile([128, 4, 512], f16, name="tt", tag="tt")
                        cnt = [1] * 4
                        # fp8 DoubleRow gates
                        for kp in range(4):
                            for wsb_, gates in (
                                (w_sb, f8_gates),
                                (wr_sb, WRES_GATES) if any_wres else (None, ()),
                            ):
                                for g in gates:
                                    for q in range(2):
                                        c0_ = 1024 * g + 512 * hh + 256 * q
                                        cnt[g] += 1
                                        nc.tensor.matmul(
                                            ps[:, BANK[g], 256 * q : 256 * q + 256],
                                            ht8[rd][:, bc, kp],
                                            wsb_[:, kp, :, c0_ : c0_ + 256],
                                            start=False,
                                            stop=(cnt[g] == n_mm[g]),
                                            perf_mode=DR,
                                        )
                            for g in HRES_GATES:
                                for q in range(2):
                                    c0_ = 1024 * g + 512 * hh + 256 * q
                                    cnt[g] += 1
                                    nc.tensor.matmul(
                                        ps[:, BANK[g], 256 * q : 256 * q + 256],
                                        rt8[rd][:, bc, kp],
                                        w_sb[:, kp, :, c0_ : c0_ + 256],
                                        start=False,
                                        stop=(cnt[g] == n_mm[g]),
                                        perf_mode=DR,
                                    )
                        # fp16 gates
                        for kc in range(8):
                            for gi, g in enumerate(F16_GATES):
                                c16 = 1024 * gi + 512 * hh
                                cnt[g] += 1
                                nc.tensor.matmul(
                                    ps[:, BANK[g], :],
                                    ht16[rd][:, bc, kc // 2, kc % 2, :],
                                    w16_sb[:, kc, c16 : c16 + 512],
                                    start=False,
                                    stop=(cnt[g] == n_mm[g]),
                                )
                        # gate tanh in two 2-bank halves (banks order
                        # i,g | f,o): the u STT fires after the first half,
                        # and the next pair's x-parts can claim freed banks
                        # sooner
                        nc.scalar.activation(
                            tt[:, 0:2], ps[:, 0:2], AFT.Tanh, scale=1.0 / S
                        )
                        nc.scalar.activation(
                            tt[:, 2:4], ps[:, 2:4], AFT.Tanh, scale=1.0 / S
                        )
                        u = upool.tile([128, 512], f16, name="u", tag="u")
                        v = upool.tile([128, 512], f16, name="v", tag="v")
                        th = upool.tile([128, 512], f16, name="th", tag="th")
                        csl = c2[:, bc, 512 * hh : 512 * hh + 512]
                        # u = (t_i + 1) * t_g ; v = (t_f + 1) * c2 ; c2' = v/2 + u
                        nc.vector.scalar_tensor_tensor(
                            u, tt[:, 0], 1.0, tt[:, 1], ALU.add, ALU.mult
                        )
                        nc.vector.scalar_tensor_tensor(
                            v, tt[:, 2], 1.0, csl, ALU.add, ALU.mult
                        )
                        nc.vector.scalar_tensor_tensor(
                            csl, v, 0.5, u, ALU.mult, ALU.add
                        )
                        # th = tanh(c) = tanh(0.5 * c2') ; h' = (t_o + 1) * th
                        nc.scalar.activation(th, csl, AFT.Tanh, scale=0.5)
                        nc.vector.scalar_tensor_tensor(
                            h16[:, bc, 512 * hh : 512 * hh + 512],
                            tt[:, 3],
                            1.0,
                            th,
                            ALU.add,
                            ALU.mult,
                        )  # o sits in bank 3
                        if t < T - 1:
                            # h'^T via one XBAR DMA per group (tile-major
                            # 128x128 blocks land exactly on the [kp,i,b']
                            # layout); fp8 value (+ residual) on Pool
                            nc.sync.dma_start(
                                ht16[wr][:, bc, 2 * hh : 2 * hh + 2, :, :],
                                h16[:, bc, 512 * hh : 512 * hh + 512],
                                transpose=True,
                            )
                            tsl = (slice(None), bc, slice(2 * hh, 2 * hh + 2))
                            # fp8 cast on ACT (Pool's copy runs ~4x slower
                            # and sat in the recurrence tail)
                            nc.scalar.activation(
                                ht8[wr][tsl], ht16[wr][tsl], AFT.Copy
                            )
                            if any_hres:
                                nc.gpsimd.tensor_sub(
                                    rt8[wr][tsl], ht16[wr][tsl], ht8[wr][tsl]
                                )

                if t == T - 1:
                    nc.sync.dma_start(out_dram[:], h16[:])

    nc.compile()
    return nc


def _host_prep(inputs):
    obs = np.asarray(inputs["obs_traj"], dtype=np.float32)
    h0 = np.asarray(inputs["h0"], dtype=np.float32)
    c0 = np.asarray(inputs["c0"], dtype=np.float32)
    W_emb = np.asarray(inputs["W_emb"], dtype=np.float32)
    b_emb = np.asarray(inputs["b_emb"], dtype=np.float32)
    W_ih = np.asarray(inputs["W_ih"], dtype=np.float32)
    W_hh = np.asarray(inputs["W_hh"], dtype=np.float32)
    b_ih = np.asarray(inputs["b_ih"], dtype=np.float32)
    b_hh = np.asarray(inputs["b_hh"], dtype=np.float32)

    gs = np.full(4 * H, 0.5, np.float32)  # sigma gates get the tanh half-angle
    gs[2 * H : 3 * H] = 1.0
    Wf = (W_ih @ W_emb).astype(np.float32)  # [4096, 2]
    biasf = (W_ih @ b_emb + b_ih + b_hh).astype(np.float32)

    # packed recurrent weights (psum = S * gate_mod; h' = 2h)
    Wp = S * gs[:, None] * W_hh / 2.0  # [4096, 1024]
    W8 = Wp.astype(E4M3)
    Wr8 = (Wp - W8.astype(np.float32)).astype(E4M3)

    def pack_w(Wq):  # [4096,1024] -> [4,2,128,4096]: [kp,i,p,col]
        return np.ascontiguousarray(Wq.T.reshape(4, 2, 128, 4096))

    wpk = pack_w(W8)
    wrk = pack_w(Wr8)
    if N16:
        cols16 = np.concatenate(
            [np.arange(1024 * g, 1024 * g + 1024) for g in F16_GATES]
        )
        # [kc, p, col16] = W16[col, 128*kc+p]
        w16k = np.ascontiguousarray(
            Wp[cols16].astype(np.float16).T.reshape(8, 128, 1024 * N16)
        )

    wfold = np.zeros((128, 4096), np.float16)
    for r in range(4):
        wfold[32 * r + 0] = (S * gs * Wf[:, 0]).astype(np.float16)
        wfold[32 * r + 1] = (S * gs * Wf[:, 1]).astype(np.float16)
        wfold[32 * r + 2] = (S * gs * biasf).astype(np.float16)

    def pack_hT(x):  # [256,1024] f32 -> [128,2,4,2,128]: [p,bc,kp,i,b']
        return np.ascontiguousarray(
            x.T.reshape(4, 2, 128, 2, 128).transpose(2, 3, 0, 1, 4)
        )

    in_maps = []
    for c in range(NCORES):
        sl = slice(B * c, B * (c + 1))
        obs_aug = np.zeros((T, 128, B), np.float16)
        for r in range(4):
            obs_aug[:, 32 * r + 0] = obs[:, sl, 0]
            obs_aug[:, 32 * r + 1] = obs[:, sl, 1]
            obs_aug[:, 32 * r + 2] = 1.0
        hp = pack_hT(2.0 * h0[sl])
        h16t = hp.astype(np.float16)
        h8t = h16t.astype(E4M3)  # device quantizes fp16 h, match that
        r8t = (h16t.astype(np.float32) - h8t.astype(np.float32)).astype(E4M3)
        c0t = np.ascontiguousarray(
            (2.0 * c0[sl]).reshape(2, 128, 1024).transpose(1, 0, 2)
        ).astype(np.float16)
        m = dict(wpk=wpk, wfold=wfold, obs=obs_aug, h16t=h16t, h8t=h8t, c0t=c0t)
        if len(WRES_GATES) > 0:
            m["wrk"] = wrk
        if len(HRES_GATES) > 0:
            m["r8t"] = r8t
        if N16:
            m["w16k"] = w16k
        in_maps.append(m)
    return in_maps


def _unpack_out(o):  # [128,2,1024] f16 -> [256,1024] f32 (h = h'/2)
    return 0.5 * o.astype(np.float32).transpose(1, 0, 2).reshape(B, H)


def kernel(**inputs) -> np.ndarray:
    global _CACHED_NC, LAST_RESULT
    from concourse.bass_utils import run_bass_kernel_spmd

    in_maps = _host_prep(inputs)
    if _CACHED_NC is None:
        _CACHED_NC = _build_program()
        _dedupe_ldweights(_CACHED_NC)
    res = run_bass_kernel_spmd(_CACHED_NC, in_maps, core_ids=list(range(NCORES)))
    LAST_RESULT = res
    out = np.concatenate(
        [_unpack_out(res.results[c]["hout"]) for c in range(NCORES)], axis=0
    )
    return np.ascontiguousarray(out.astype(np.float32))
